# revision 1
# baseline (speedup 1.0000x reference)
"""Trainium2 Bass kernel for nn_EventSequenceEmbedder.

Strategy
--------
The whole module is algebraically folded on the host into a single small
matrix product per token:

    out[t, :] = featT[:, t] . M  (masked)

where
  * M [104, 256] is built once from the weights: each embedding table and
    each linear projection is folded through its combine_W column block
    (pure weight preprocessing), all biases collapse into one bias row.
  * featT [104, BS] is the per-token sparse feature vector:
      rows 0:53    card multihot (counts of the 7 card ids; /7 folded into M)
      rows 53:62   hero one-hot
      rows 62:71   acting one-hot
      rows 71:81   num_players one-hot
      rows 81:102  raw numeric features (scalars2, blinds2, bets9, action8)
      row  102     ones (bias row)
      row  103     zero padding
    The whole featT is scaled by mask, which reproduces `out * mask` exactly.

Sharding: data-parallel over tokens. B*S = 32768 tokens are split into 8
contiguous blocks of 4096; each NeuronCore computes out = featT_blk.T @ M
as 32 PE matmuls (lhsT = featT chunk [104,128] fp16, rhs = M [104,256] fp16,
fp32 PSUM), drains PSUM pairs via alternating Vector/Scalar engines into
fp16 output tiles, and DMAs them back to DRAM on alternating HWDGE rings
(the host upcasts to f32). Drains stay pair-granular (one PSUM bank)
for deep pipelining, but output DMAs are quad-grouped: 4 chunks per
staging tile / 8 DMAs total (per-dma_start ring-issue cost ~0.8us makes
DMA count a first-class knob). Token order is permuted on the host
(row = g*512 + 4p + j) so every output descriptor is 2KB contiguous.

Memory-roofline bound: ~2.9 MB of HBM traffic per core (852KB featT in,
2MB fp16 out) at the ~210-230 GB/s practical per-core DMA rate.
Measured via on-device For_i repetition slope (no NTFF profiling under
axon): ~13.4 us per full pass in clean windows (session noise up to
~17); decomposition: in-DMA ~6-7us, out-DMA ~6-9us, PE+drains ~8us,
overlapped. Numerical error vs the fp32 reference: ~5e-4 max rel
(fp16 features/weights/output, fp32 PSUM accumulation).
"""

import os

import ml_dtypes
import numpy as np

import concourse.bass as bass
import concourse.mybir as mybir
import concourse.tile as tile
from concourse import bacc
from concourse.bass_utils import run_bass_kernel_spmd

# NTFF tracing is unavailable under axon (antenv.axon_hooks absent) —
# force it off so a stray BASS_TRACE=1 in the environment can't crash us.
os.environ["BASS_NEVER_TRACE"] = "1"

# Problem shape (hardcoded per harness contract)
B, S, D, MP, NA, NCARDS = 32, 1024, 256, 9, 8, 53
BS = B * S            # 32768 tokens
NCORES = 8
TOK = BS // NCORES    # 4096 tokens per core
KF = 104              # feature rows: 81 onehot + 21 numeric + 1 bias + 1 pad
NPAIR = TOK // 256    # 16 psum-bank iterations (2 chunks of 128 tokens each)

_CACHE = {}
LAST_RESULT = None    # BassKernelResults of the most recent run (for profiling)


OUTG = 4              # chunks per output DMA (quad grouping)


def _token_perm():
    """featT column order: column (G*g + j)*128 + p holds token
    g*(128*G) + G*p + j, so each output staging tile writes G consecutive
    DRAM rows per partition (2KB contiguous fp16 descriptors) and one
    DMA covers G chunks."""
    if "perm" not in _CACHE:
        i = np.arange(TOK)
        c, p = i // 128, i % 128
        g, j = c // OUTG, c % OUTG
        _CACHE["perm"] = g * (128 * OUTG) + OUTG * p + j
    return _CACHE["perm"]


def _build_program(reps=None, out_dtype="float16", perm_layout=True,
                   staggered=False):
    """Build + compile the per-core Bass program (identical on all cores).

    reps: if set, wrap the whole body in an on-device For_i loop that
    repeats the full workload (input DMA + matmuls + drains + output DMA)
    `reps` times — used only for timing (wall-clock slope over reps).
    """
    odt = getattr(mybir.dt, out_dtype)
    nc = bacc.Bacc("TRN2", target_bir_lowering=False, debug=False,
                   num_devices=NCORES)
    featT_d = nc.dram_tensor("featT", [KF, TOK], mybir.dt.float16,
                             kind="ExternalInput")
    m_d = nc.dram_tensor("mcomb", [KF, D], mybir.dt.float16,
                         kind="ExternalInput")
    out_d = nc.dram_tensor("out", [TOK, D], odt, kind="ExternalOutput")

    with tile.TileContext(nc) as tc:
        with (
            tc.tile_pool(name="consts", bufs=2) as cpool,
            tc.tile_pool(name="psum", bufs=8, space="PSUM") as ppool,
            tc.tile_pool(name="outs", bufs=6) as opool,
        ):
            def body(_i=None):
                m_t = cpool.tile([KF, D], mybir.dt.float16, tag="mtile")
                # load M on the scalar ring so it runs parallel with featT
                # slice 0 (sync ring) — chunk-0 matmuls need both
                nc.scalar.dma_start(m_t[:], m_d[:])
                f_t = cpool.tile([KF, TOK], mybir.dt.float16, tag="ftile")
                # split the featT load into 6 DMAs alternating across the
                # two HWDGE rings (sync=SP, scalar=ACT); progressively
                # sized slices (small first) let the first matmul chunks
                # launch ~0.5us earlier, trimming pipeline ramp
                pos = 0
                for i, w in enumerate((256, 512, 832, 832, 832, 832)):
                    eng = nc.sync if i % 2 == 0 else nc.scalar
                    eng.dma_start(f_t[:, pos:pos + w], featT_d[:, pos:pos + w])
                    pos += w

                # Token order is permuted on the host (see _token_perm) so
                # each staging tile writes OUTG consecutive DRAM rows per
                # partition (2KB contiguous fp16 descriptors) and one DMA
                # covers OUTG chunks. Drains stay pair-granular (one PSUM
                # bank) for pipelining; output DMAs alternate HWDGE rings.
                G = OUTG
                out_v = out_d[:].rearrange("(g p j) d -> g p j d",
                                           j=G, p=128)
                for g in range(32 // G):
                    stage = opool.tile([128, G, D], odt)
                    for pb in range(G // 2):
                        bb = g * (G // 2) + pb
                        ps = ppool.tile([128, 2, D], mybir.dt.float32)
                        for h in range(2):
                            c = 2 * bb + h
                            nc.tensor.matmul(ps[:, h, :],
                                             f_t[:, c * 128:(c + 1) * 128],
                                             m_t[:], start=True, stop=True)
                        if bb % 2 == 0:
                            nc.vector.tensor_copy(
                                stage[:, pb * 2:(pb + 1) * 2, :], ps[:])
                        else:
                            nc.scalar.copy(
                                stage[:, pb * 2:(pb + 1) * 2, :], ps[:])
                    eng = nc.sync if g % 2 == 0 else nc.scalar
                    eng.dma_start(out_v[g], stage[:])

            if reps is None:
                body()
            else:
                with tc.For_i(0, reps, 1, staggered_reset=staggered):
                    body()

    nc.compile()
    return nc


def _fold_weights(card_table, hero_table, acting_table, nump_table,
                  scalar_W, scalar_b, blind_W, blind_b, bet_W, bet_b,
                  action_W, action_b, combine_W, combine_b):
    """Fold all tables/projections through combine_W into M [104, D] (fp32)."""
    W = np.asarray(combine_W, np.float32)          # [D, 8D]
    blk = [W[:, k * D:(k + 1) * D] for k in range(8)]
    # concat order: card, hero, acting, scalar, bet, action, nump, blind
    Wcard, Where, Wact, Wscal, Wbet, Waction, Wnump, Wblind = blk
    M = np.zeros((KF, D), np.float32)
    M[0:53] = np.asarray(card_table, np.float32) @ Wcard.T / 7.0
    M[53:62] = np.asarray(hero_table, np.float32) @ Where.T
    M[62:71] = np.asarray(acting_table, np.float32) @ Wact.T
    M[71:81] = np.asarray(nump_table, np.float32) @ Wnump.T
    M[81:83] = (Wscal @ np.asarray(scalar_W, np.float32)).T
    M[83:85] = (Wblind @ np.asarray(blind_W, np.float32)).T
    M[85:94] = (Wbet @ np.asarray(bet_W, np.float32)).T
    M[94:102] = (Waction @ np.asarray(action_W, np.float32)).T
    M[102] = (np.asarray(combine_b, np.float32)
              + Wscal @ np.asarray(scalar_b, np.float32)
              + Wblind @ np.asarray(blind_b, np.float32)
              + Wbet @ np.asarray(bet_b, np.float32)
              + Waction @ np.asarray(action_b, np.float32))
    return M


def _build_features(cards, hero_pos, acting_pos, num_players,
                    scalars, blinds, bets, action, mask):
    """Build featT [104, BS] fp32 (mask folded in)."""
    cards = np.asarray(cards).reshape(BS, 7).astype(np.int64)
    hero = np.asarray(hero_pos).reshape(BS).astype(np.int64)
    act = np.asarray(acting_pos).reshape(BS).astype(np.int64)
    nump = np.asarray(num_players).reshape(BS).astype(np.int64)
    msk = np.asarray(mask, np.float32).reshape(BS)

    feat = np.zeros((BS, KF), np.float32)
    ar53 = np.arange(NCARDS, dtype=np.int64)
    feat[:, 0:53] = (cards[:, :, None] == ar53).sum(axis=1, dtype=np.float32)
    feat[:, 53:62] = hero[:, None] == np.arange(9)
    feat[:, 62:71] = act[:, None] == np.arange(9)
    feat[:, 71:81] = nump[:, None] == np.arange(10)
    num = np.concatenate([
        np.asarray(scalars, np.float32).reshape(BS, 2),
        np.asarray(blinds, np.float32).reshape(BS, 2),
        np.asarray(bets, np.float32).reshape(BS, MP),
        np.asarray(action, np.float32).reshape(BS, NA),
    ], axis=1) * msk[:, None]
    feat[:, 81:102] = num          # num already carries the mask
    feat[:, 102] = msk             # bias row (masked)
    feat[:, 0:81] *= msk[:, None]
    return feat.T


def kernel(cards, hero_pos, acting_pos, num_players, scalars, blinds, bets,
           action, mask, card_table, hero_table, acting_table, nump_table,
           scalar_W, scalar_b, blind_W, blind_b, bet_W, bet_b,
           action_W, action_b, combine_W, combine_b):
    global LAST_RESULT
    if "nc" not in _CACHE:
        _CACHE["nc"] = _build_program()
    nc = _CACHE["nc"]

    M = _fold_weights(card_table, hero_table, acting_table, nump_table,
                      scalar_W, scalar_b, blind_W, blind_b, bet_W, bet_b,
                      action_W, action_b, combine_W, combine_b)
    featT = _build_features(cards, hero_pos, acting_pos, num_players,
                            scalars, blinds, bets, action, mask)

    m16 = np.ascontiguousarray(M, dtype=np.float16)
    in_maps = []
    for i in range(NCORES):
        f16 = np.ascontiguousarray(
            featT[:, i * TOK:(i + 1) * TOK],
            dtype=np.float16)[:, _token_perm()]
        in_maps.append({"featT": np.ascontiguousarray(f16), "mcomb": m16})

    res = run_bass_kernel_spmd(nc, in_maps, core_ids=list(range(NCORES)))
    LAST_RESULT = res
    out = np.concatenate([res.results[i]["out"] for i in range(NCORES)],
                         axis=0).astype(np.float32)
    return out.reshape(B, S, D)

